# revision 1
# baseline (speedup 1.0000x reference)
"""Trainium2 Bass kernel for NCM/kNN retrieval (nn_NCM_30468497998426).

reference computation:
    mean-center support [C=1000,S=5,D=512] and queries [Q=5000,D=512] by the
    support mean, L2-normalize, sims = einsum('csd,qd->cqs'), max over shots,
    argmax over classes -> [Q] int32.

Sharding: queries split across 8 cores (625 each), support replicated.
Per core (pipelined so the fp32 main matmul overlaps preprocessing):
    A     = support.reshape(5000, 512), 40 natural tiles [125, 512]
    mu    = colsum(A) via DVE add tree + one ones-matmul, then /5000
    shat  = (A - mu) / ||A - mu||  rowwise (DVE sub, ACT square+scale)
    ShatT = PE transpose -> 4x[128, 5000];  qcT likewise (queries centered,
            not normalized: a positive per-query scale cannot move the argmax)
    per cs-chunk j: sims^T[q, 500cs] = qcT.T @ ShatT[:, j]  (fp32, exact)
                    best[q, 100c]    = strided max over shots out of PSUM
    out[q] = argmax_c best  (DVE max_with_indices)
"""

import numpy as np

import concourse.bacc as bacc
import concourse.mybir as mybir
import concourse.tile as tile
from concourse.alu_op_type import AluOpType
from concourse.bass_utils import run_bass_kernel_spmd

F32 = mybir.dt.float32
I32 = mybir.dt.int32
U32 = mybir.dt.uint32
AF = mybir.ActivationFunctionType

C, S, D = 1000, 5, 512
CS = C * S              # 5000 support rows
Q = 5000
NCORES = 8
QS = Q // NCORES        # 625 queries per core
P = 125                 # rows per natural tile
NT = CS // P            # 40 support tiles
KC = D // 128           # 4 contraction chunks
QT = QS // P            # 5 query tiles
CSCH = 500              # cs per PSUM chunk
NJ = CS // CSCH         # 10 cs chunks (4 support tiles each)
TPJ = CSCH // P         # support tiles per chunk (4)
GPC = CSCH // S         # classes per chunk (100)


def build():
    nc = bacc.Bacc(None, target_bir_lowering=False)

    sup = nc.declare_dram_parameter("support", [CS, D], F32, isOutput=False)
    qry = nc.declare_dram_parameter("queries", [QS, D], F32, isOutput=False)
    ident = nc.declare_dram_parameter("ident", [128, 128], F32, isOutput=False)
    ones_col = nc.declare_dram_parameter("ones_col", [128, 1], F32, isOutput=False)
    ones_row = nc.declare_dram_parameter("ones_row", [1, 128], F32, isOutput=False)
    out = nc.declare_dram_parameter("out", [QS, 1], I32, isOutput=True)

    flip = [0]

    def copyback(dst, src):
        if flip[0] % 2 == 0:
            nc.vector.tensor_copy(dst, src)
        else:
            nc.scalar.copy(dst, src)
        flip[0] += 1

    with tile.TileContext(nc) as tc:
        with (
            tc.tile_pool(name="const", bufs=1) as pconst,
            tc.tile_pool(name="stat", bufs=1) as pstat,
            tc.tile_pool(name="st", bufs=NJ) as pst,
            tc.tile_pool(name="qt", bufs=1) as pqt,
            tc.tile_pool(name="anat", bufs=8) as pa,
            tc.tile_pool(name="scratch", bufs=2) as pscr,
            tc.tile_pool(name="rows", bufs=NT) as prows,
            tc.tile_pool(name="best", bufs=1) as pbest,
            tc.tile_pool(name="res", bufs=2) as pres,
            tc.tile_pool(name="trpsum", bufs=1, space="PSUM") as ptr,
            tc.tile_pool(name="mmpsum", bufs=1, space="PSUM") as pmm,
        ):
            id_sb = pconst.tile([128, 128], F32, tag="ident")
            nc.sync.dma_start(id_sb[:], ident[:])
            onec_sb = pconst.tile([128, 1], F32, tag="onec")
            nc.sync.dma_start(onec_sb[:], ones_col[:])
            oner_sb = pconst.tile([1, 128], F32, tag="oner")
            nc.sync.dma_start(oner_sb[:], ones_row[:])

            qt_tiles = [pqt.tile([128, QS], F32, name=f"qt{k}", tag=f"qt{k}")
                        for k in range(KC)]

            # ---- loads (queries first: small, unblock the q side early)
            inner = tc.tile_pool(name="qnat", bufs=QT)
            pq = inner.__enter__()
            macc_cm = tc.tile_pool(name="macc", bufs=1)
            pmacc = macc_cm.__enter__()
            with nc.named_scope("load"):
                q_tiles = []
                for i in range(QT):
                    qt_ = pq.tile([P, D], F32, tag="q", bufs=QT)
                    nc.sync.dma_start(qt_[:], qry[i * P:(i + 1) * P, :])
                    q_tiles.append(qt_)

            # ---- mean: stream support once, DVE add tree (keeps PE free)
            with nc.named_scope("mean"):
                NG = 4
                gacc = []
                for g in range(NG):
                    acc = pmacc.tile([P, D], F32, tag=f"acc{g}", name=f"acc{g}")
                    nc.sync.dma_start(acc[:], sup[g * P:(g + 1) * P, :])
                    gacc.append(acc)
                for t in range(NG, NT):
                    lt = pmacc.tile([P, D], F32, tag="ld", bufs=6)
                    nc.sync.dma_start(lt[:], sup[t * P:(t + 1) * P, :])
                    nc.vector.tensor_add(gacc[t % NG][:], gacc[t % NG][:],
                                         lt[:])
                for step in (2, 1):
                    for g in range(step):
                        nc.vector.tensor_add(gacc[g][:], gacc[g][:],
                                             gacc[g + step][:])
                mu_ps = ptr.tile([1, D], F32, tag="mu", bufs=1)
                nc.tensor.matmul(mu_ps[:], onec_sb[0:P, :], gacc[0][:],
                                 start=True, stop=True)
                mu_sb = pstat.tile([1, D], F32, tag="mu_sb")
                nc.vector.tensor_scalar_mul(mu_sb[:], mu_ps[:], 1.0 / CS)
                mub_ps = ptr.tile([128, D], F32, tag="mub", bufs=1)
                nc.tensor.matmul(mub_ps[:], oner_sb[:], mu_sb[:],
                                 start=True, stop=True)
                mu_b = pstat.tile([128, D], F32, tag="mu_b")
                nc.vector.tensor_copy(mu_b[:], mub_ps[:])

            # ---- query side: center, transpose, copy back
            with nc.named_scope("qside"):
                for i in range(QT):
                    qt_ = q_tiles[i]
                    nc.vector.tensor_sub(qt_[:], qt_[:], mu_b[0:P, :])
                    for k in range(KC):
                        tp = ptr.tile([128, P], F32, tag="tp", bufs=3)
                        nc.tensor.transpose(tp[:],
                                            qt_[:, k * 128:(k + 1) * 128],
                                            id_sb[0:P, 0:P])
                        copyback(qt_tiles[k][:, i * P:(i + 1) * P], tp[:])

            macc_cm.__exit__(None, None, None)
            inner.__exit__(None, None, None)

            # ---- pipelined: per cs-chunk preprocess 4 tiles, then matmul
            best_tiles = [pbest.tile([P, C], F32, name=f"best{i}", tag=f"best{i}")
                          for i in range(QT)]
            for j in range(NJ):
                stj = [pst.tile([128, CSCH], F32, name=f"st{k}_{j}", tag=f"st{k}")
                       for k in range(KC)]
                with nc.named_scope(f"prep{j}"):
                    for tt in range(TPJ):
                        t = j * TPJ + tt
                        at = pa.tile([P, D], F32, tag="a")
                        nc.sync.dma_start(at[:], sup[t * P:(t + 1) * P, :])
                        nc.vector.tensor_sub(at[:], at[:], mu_b[0:P, :])
                        sq = pscr.tile([P, D], F32, tag="sq")
                        n2 = prows.tile([P, 1], F32, tag="n2")
                        nc.scalar.activation(sq[:], at[:], AF.Square,
                                             accum_out=n2[:])
                        nrm = prows.tile([P, 1], F32, tag="nrm")
                        nc.scalar.activation(nrm[:], n2[:], AF.Sqrt)
                        inv = prows.tile([P, 1], F32, tag="inv")
                        nc.vector.reciprocal(inv[:], nrm[:])
                        nc.scalar.activation(at[:], at[:], AF.Copy,
                                             scale=inv[:])
                        for k in range(KC):
                            tp = ptr.tile([128, P], F32, tag="tp", bufs=3)
                            nc.tensor.transpose(
                                tp[:], at[:, k * 128:(k + 1) * 128],
                                id_sb[0:P, 0:P])
                            copyback(stj[k][:, tt * P:(tt + 1) * P], tp[:])
                with nc.named_scope(f"mm{j}"):
                    for i in range(QT):
                        ps = pmm.tile([P, CSCH], F32, tag="sims", bufs=3)
                        for k in range(KC):
                            nc.tensor.matmul(
                                ps[:],
                                qt_tiles[k][:, i * P:(i + 1) * P],
                                stj[k][:, :],
                                start=(k == 0), stop=(k == KC - 1),
                            )
                        nc.vector.tensor_reduce(
                            out=best_tiles[i][:, j * GPC:(j + 1) * GPC],
                            in_=ps[:].rearrange("p (c s) -> p c s", s=S),
                            axis=mybir.AxisListType.X, op=AluOpType.max,
                        )

            # ---- argmax over classes
            with nc.named_scope("argmax"):
                for i in range(QT):
                    mx8 = pres.tile([P, 8], F32, tag="mx8")
                    ix8 = pres.tile([P, 8], U32, tag="ix8")
                    nc.vector.max_with_indices(mx8[:], ix8[:], best_tiles[i][:])
                    ii = pres.tile([P, 1], I32, tag="ii")
                    nc.vector.tensor_copy(ii[:], ix8[:, 0:1])
                    nc.sync.dma_start(out[i * P:(i + 1) * P, :], ii[:])

    nc.finalize()
    return nc


def _host_inputs(support_features, query_features):
    sup = np.ascontiguousarray(
        np.asarray(support_features, dtype=np.float32).reshape(CS, D))
    qf = np.ascontiguousarray(np.asarray(query_features, dtype=np.float32))
    ident = np.eye(128, dtype=np.float32)
    ones_col = np.ones((128, 1), dtype=np.float32)
    ones_row = np.ones((1, 128), dtype=np.float32)
    in_maps = []
    for c in range(NCORES):
        in_maps.append({
            "support": sup,
            "queries": np.ascontiguousarray(qf[c * QS:(c + 1) * QS]),
            "ident": ident,
            "ones_col": ones_col,
            "ones_row": ones_row,
        })
    return in_maps


def run(support_features, query_features, trace=False, **trace_kwargs):
    nc = build()
    in_maps = _host_inputs(support_features, query_features)
    res = run_bass_kernel_spmd(nc, in_maps, list(range(NCORES)),
                               trace=trace, **trace_kwargs)
    outs = [np.asarray(r["out"]).reshape(QS) for r in res.results]
    return np.concatenate(outs).astype(np.int32), res


def kernel(support_features, query_features, use_cosine=None, **_ignored):
    # use_cosine does not change the result: with L2-normalized vectors the
    # euclidean argmin equals the cosine argmax (monotone map), so one kernel
    # serves both branches.
    out, _ = run(support_features, query_features, trace=False)
    return out



# revision 11
# speedup vs baseline: 1.3502x; 1.3502x over previous
"""Trainium2 Bass kernel for NCM/kNN retrieval (nn_NCM_30468497998426).

reference computation:
    mean-center support [C=1000,S=5,D=512] and queries [Q=5000,D=512] by the
    support mean, L2-normalize, sims = einsum('csd,qd->cqs'), max over shots,
    argmax over classes -> [Q] int32.

Sharding: queries split across 8 cores (625 each), support replicated.

v2 layout: support is HOST-transposed to [512, 5000] (pure layout change),
so the kernel needs NO PE transposes or PSUM->SBUF copybacks at all:
    - 4 big DMAs load support d-major chunks [128, 5000]; queries arrive
      host-transposed+zero-padded as [512, 640].
    - mean: chunked free-dim reduces (DVE tensor_reduce / ACT accum split)
      pipelined under the DMA -> mu_T [128,1] per d-chunk.
    - queries centered with per-partition mu_T, rounded to f32r.
    - per cs-chunk j (500 cols): ACT Square(x + bias=-mu) emits squared
      centered values (rounded to f32r), f32r ones-matmul column-sums them
      in PSUM -> |s-mu|^2 replicated across partitions; ACT Sqrt + DVE
      reciprocal -> inv_rep [128,500]; one fused scalar_tensor_tensor pass
      (x - mu)*inv_rep produces the normalized support chunk in f32r.
    - mains: 4 f32r accumulate-matmuls -> PSUM sims [128q, 500cs], DVE max
      over shots -> best[q, 100c]; final argmax via MAX8.
"""

import numpy as np

import concourse.bacc as bacc
import concourse.mybir as mybir
import concourse.tile as tile
from concourse.alu_op_type import AluOpType
from concourse.bass_utils import run_bass_kernel_spmd

F32 = mybir.dt.float32
F32R = mybir.dt.float32r
I32 = mybir.dt.int32
U32 = mybir.dt.uint32
BF16 = mybir.dt.bfloat16
AF = mybir.ActivationFunctionType

C, S, D = 1000, 5, 512
CS = C * S              # 5000 support rows
Q = 5000
NCORES = 8
QS = Q // NCORES        # 625 queries per core
QSP = 640               # padded to 5x128 (f32r wants even/128-wide tiles)
PW = 128                # queries per stationary tile
KC = D // 128           # 4 contraction chunks
QT = QSP // PW          # 5 query tiles
CSCH = 500              # cs per PSUM chunk
NJ = CS // CSCH         # 10 cs chunks
GPC = CSCH // S         # classes per chunk (100)
MR = 8                  # mean sub-reduces per d-chunk
MW = CS // MR           # 625 cols per mean sub-reduce


def build():
    nc = bacc.Bacc(None, target_bir_lowering=False)

    sup_t = nc.declare_dram_parameter("support_t", [D, CS], F32, isOutput=False)
    qry_t = nc.declare_dram_parameter("queries_t", [D, QSP], F32, isOutput=False)
    ones_cr = nc.declare_dram_parameter("ones_cr", [128, 128], F32, isOutput=False)
    out = nc.declare_dram_parameter("out", [QS, 1], I32, isOutput=True)

    def r(ap):
        return ap.bitcast(F32R)

    with tile.TileContext(nc) as tc:
        with (
            tc.tile_pool(name="const", bufs=1) as pconst,
            tc.tile_pool(name="stat", bufs=1) as pstat,
            tc.tile_pool(name="sраw", bufs=1) as praw,
            tc.tile_pool(name="st", bufs=2 * KC) as pst,
            tc.tile_pool(name="qt", bufs=1) as pqt,
            tc.tile_pool(name="qc", bufs=1) as pqc,
            tc.tile_pool(name="sq", bufs=3) as psq,
            tc.tile_pool(name="nrm", bufs=2) as pnrm,
            tc.tile_pool(name="best", bufs=1) as pbest,
            tc.tile_pool(name="res", bufs=2) as pres,
            tc.tile_pool(name="n2psum", bufs=2, space="PSUM") as pn2,
            tc.tile_pool(name="mmpsum", bufs=3, space="PSUM") as pmm,
        ):
            ones_sb = pconst.tile([128, 128], F32, tag="ones")
            nc.sync.dma_start(ones_sb[:], ones_cr[:])

            # ---- loads: queries first (small), then 4 support d-chunks
            qt_tiles = []
            with nc.named_scope("load_q"):
                for k in range(KC):
                    qt_ = pqt.tile([128, QSP], F32, name=f"qt{k}", tag=f"qt{k}")
                    nc.sync.dma_start(qt_[:], qry_t[k * 128:(k + 1) * 128, :])
                    qt_tiles.append(qt_)
            st_raw = []
            with nc.named_scope("load_s"):
                for k in range(KC):
                    st_ = praw.tile([128, CS], F32, name=f"sraw{k}", tag=f"sraw{k}")
                    nc.sync.dma_start(st_[:], sup_t[k * 128:(k + 1) * 128, :])
                    st_raw.append(st_)

            # ---- mean: free-dim reduces pipelined under the DMA
            with nc.named_scope("mean"):
                msub = pstat.tile([128, KC * MR], F32, tag="msub")
                for k in range(KC):
                    for m in range(MR):
                        sl = st_raw[k][:, m * MW:(m + 1) * MW]
                        col = msub[:, k * MR + m:k * MR + m + 1]
                        if m % 2 == 0:
                            nc.vector.tensor_reduce(
                                out=col, in_=sl, axis=mybir.AxisListType.X,
                                op=AluOpType.add)
                        else:
                            dump = psq.tile([128, MW], F32, tag="mdump")
                            nc.scalar.activation(dump[:], sl, AF.Copy,
                                                 accum_out=col)
                mu_t = pstat.tile([128, KC], F32, tag="mu_t")
                nmu_t = pstat.tile([128, KC], F32, tag="nmu_t")
                for k in range(KC):
                    acc = pstat.tile([128, 1], F32, tag=f"macc{k}")
                    nc.vector.tensor_reduce(
                        out=acc[:], in_=msub[:, k * MR:(k + 1) * MR],
                        axis=mybir.AxisListType.X, op=AluOpType.add)
                    nc.vector.tensor_scalar_mul(mu_t[:, k:k + 1], acc[:],
                                                1.0 / CS)
                    nc.vector.tensor_scalar_mul(nmu_t[:, k:k + 1], acc[:],
                                                -1.0 / CS)

            # ---- query side: center exactly, split into bf16 hi/lo
            qhi_tiles, qlo_tiles = [], []
            with nc.named_scope("qside"):
                for k in range(KC):
                    qc = pqc.tile([128, QSP], F32, name=f"qc{k}", tag=f"qc{k}")
                    nc.vector.tensor_scalar_sub(qc[:], qt_tiles[k][:],
                                                mu_t[:, k:k + 1])
                    qhi = pqc.tile([128, QSP], BF16, name=f"qhi{k}",
                                   tag=f"qhi{k}")
                    nc.scalar.copy(qhi[:], qc[:])
                    qlo = pqc.tile([128, QSP], BF16, name=f"qlo{k}",
                                   tag=f"qlo{k}")
                    nc.vector.tensor_sub(qlo[:], qc[:], qhi[:])
                    qhi_tiles.append(qhi)
                    qlo_tiles.append(qlo)

            # ---- pipelined per cs-chunk: norms -> normalize -> matmul
            best_tiles = [pbest.tile([PW, C], F32, name=f"best{i}", tag=f"best{i}")
                          for i in range(QT)]
            for j in range(NJ):
                cs0 = j * CSCH
                with nc.named_scope(f"prep{j}"):
                    # |s - mu|^2 via Square(x + (-mu)) then f32r colsum,
                    # replicated across all 128 partitions by the ones lhsT
                    n2_ps = pn2.tile([128, CSCH], F32, tag="n2")
                    for k in range(KC):
                        sqj = psq.tile([128, CSCH], F32, tag="sq")
                        nc.scalar.activation(
                            sqj[:], st_raw[k][:, cs0:cs0 + CSCH],
                            AF.Square, bias=nmu_t[:, k:k + 1])
                        nc.tensor.matmul(n2_ps[:], ones_sb[:], sqj[:],
                                         start=(k == 0), stop=(k == KC - 1))
                    nrm = pnrm.tile([128, CSCH], F32, tag="nrm")
                    nc.scalar.activation(nrm[:], n2_ps[:], AF.Sqrt)
                    inv = pnrm.tile([128, CSCH], F32, tag="inv")
                    nc.vector.reciprocal(inv[:], nrm[:])
                    # normalized support chunk: (x - mu) * inv, one fused
                    # pass, then split into bf16 hi/lo for exact 3-pass mains
                    shi, slo = [], []
                    for k in range(KC):
                        st_ = pst.tile([128, CSCH], F32, name=f"st{k}_{j}",
                                       tag="st")
                        nc.vector.scalar_tensor_tensor(
                            out=st_[:], in0=st_raw[k][:, cs0:cs0 + CSCH],
                            scalar=mu_t[:, k:k + 1], in1=inv[:],
                            op0=AluOpType.subtract, op1=AluOpType.mult)
                        hi = pst.tile([128, CSCH], BF16, name=f"shi{k}_{j}",
                                      tag="shi")
                        nc.scalar.copy(hi[:], st_[:])
                        lo = pst.tile([128, CSCH], BF16, name=f"slo{k}_{j}",
                                      tag="slo")
                        nc.vector.tensor_sub(lo[:], st_[:], hi[:])
                        shi.append(hi)
                        slo.append(lo)
                with nc.named_scope(f"mm{j}"):
                    for i in range(QT):
                        ps = pmm.tile([PW, CSCH], F32, tag="sims")
                        nmm = 3 * KC
                        m = 0
                        for k in range(KC):
                            qh = qhi_tiles[k][:, i * PW:(i + 1) * PW]
                            ql = qlo_tiles[k][:, i * PW:(i + 1) * PW]
                            for lhsT, rhs in ((qh, shi[k][:]),
                                              (qh, slo[k][:]),
                                              (ql, shi[k][:])):
                                nc.tensor.matmul(
                                    ps[:], lhsT, rhs,
                                    start=(m == 0), stop=(m == nmm - 1),
                                )
                                m += 1
                        nc.vector.tensor_reduce(
                            out=best_tiles[i][:, j * GPC:(j + 1) * GPC],
                            in_=ps[:].rearrange("p (c s) -> p c s", s=S),
                            axis=mybir.AxisListType.X, op=AluOpType.max,
                        )

            # ---- argmax over classes
            with nc.named_scope("argmax"):
                for i in range(QT):
                    valid = min(PW, QS - i * PW)
                    mx8 = pres.tile([PW, 8], F32, tag="mx8")
                    ix8 = pres.tile([PW, 8], U32, tag="ix8")
                    nc.vector.max_with_indices(mx8[:], ix8[:], best_tiles[i][:])
                    ii = pres.tile([PW, 1], I32, tag="ii")
                    nc.vector.tensor_copy(ii[:], ix8[:, 0:1])
                    nc.sync.dma_start(out[i * PW:i * PW + valid, :],
                                      ii[0:valid, :])

    nc.finalize()
    return nc


def _host_inputs(support_features, query_features):
    sup = np.asarray(support_features, dtype=np.float32).reshape(CS, D)
    sup_t = np.ascontiguousarray(sup.T)
    qf = np.asarray(query_features, dtype=np.float32)
    ones_cr = np.ones((128, 128), dtype=np.float32)
    in_maps = []
    for c in range(NCORES):
        qslab = np.zeros((QSP, D), dtype=np.float32)
        qslab[:QS] = qf[c * QS:(c + 1) * QS]
        in_maps.append({
            "support_t": sup_t,
            "queries_t": np.ascontiguousarray(qslab.T),
            "ones_cr": ones_cr,
        })
    return in_maps


def run(support_features, query_features, trace=False, **trace_kwargs):
    nc = build()
    in_maps = _host_inputs(support_features, query_features)
    res = run_bass_kernel_spmd(nc, in_maps, list(range(NCORES)),
                               trace=trace, **trace_kwargs)
    outs = [np.asarray(r["out"]).reshape(QS) for r in res.results]
    return np.concatenate(outs).astype(np.int32), res


def kernel(support_features, query_features, use_cosine=None, **_ignored):
    # use_cosine does not change the result: with L2-normalized vectors the
    # euclidean argmin equals the cosine argmax (monotone map), so one kernel
    # serves both branches.
    out, _ = run(support_features, query_features, trace=False)
    return out


# revision 14
# speedup vs baseline: 1.4870x; 1.1013x over previous
"""Trainium2 Bass kernel for NCM/kNN retrieval (nn_NCM_30468497998426).

reference computation:
    mean-center support [C=1000,S=5,D=512] and queries [Q=5000,D=512] by the
    support mean, L2-normalize, sims = einsum('csd,qd->cqs'), max over shots,
    argmax over classes -> [Q] int32.

Sharding: queries split across 8 cores (625 each), support replicated.

v2 layout: support is HOST-transposed to [512, 5000] (pure layout change),
so the kernel needs NO PE transposes or PSUM->SBUF copybacks at all:
    - 4 big DMAs load support d-major chunks [128, 5000]; queries arrive
      host-transposed+zero-padded as [512, 640].
    - mean: chunked free-dim reduces (DVE tensor_reduce / ACT accum split)
      pipelined under the DMA -> mu_T [128,1] per d-chunk.
    - queries centered with per-partition mu_T, rounded to f32r.
    - per cs-chunk j (500 cols): ACT Square(x + bias=-mu) emits squared
      centered values (rounded to f32r), f32r ones-matmul column-sums them
      in PSUM -> |s-mu|^2 replicated across partitions; ACT Sqrt + DVE
      reciprocal -> inv_rep [128,500]; one fused scalar_tensor_tensor pass
      (x - mu)*inv_rep produces the normalized support chunk in f32r.
    - mains: 4 f32r accumulate-matmuls -> PSUM sims [128q, 500cs], DVE max
      over shots -> best[q, 100c]; final argmax via MAX8.
"""

import numpy as np

import concourse.bacc as bacc
import concourse.mybir as mybir
import concourse.tile as tile
from concourse.alu_op_type import AluOpType
from concourse.bass_utils import run_bass_kernel_spmd

F32 = mybir.dt.float32
F32R = mybir.dt.float32r
I32 = mybir.dt.int32
U32 = mybir.dt.uint32
BF16 = mybir.dt.bfloat16
AF = mybir.ActivationFunctionType

C, S, D = 1000, 5, 512
CS = C * S              # 5000 support rows
Q = 5000
NCORES = 8
QS = Q // NCORES        # 625 queries per core
QSP = 640               # padded to 5x128 (f32r wants even/128-wide tiles)
PW = 128                # queries per stationary tile
KC = D // 128           # 4 contraction chunks
QT = QSP // PW          # 5 query tiles
CSCH = 500              # cs per PSUM chunk
NJ = CS // CSCH         # 10 cs chunks
GPC = CSCH // S         # classes per chunk (100)
MR = 8                  # mean sub-reduces per d-chunk
MW = CS // MR           # 625 cols per mean sub-reduce


def build():
    nc = bacc.Bacc(None, target_bir_lowering=False)

    sup_t = nc.declare_dram_parameter("support_t", [D, CS], F32, isOutput=False)
    qry_t = nc.declare_dram_parameter("queries_t", [D, QSP], F32, isOutput=False)
    ones_cr = nc.declare_dram_parameter("ones_cr", [128, 128], F32, isOutput=False)
    out = nc.declare_dram_parameter("out", [QS, 1], I32, isOutput=True)

    def r(ap):
        return ap.bitcast(F32R)

    with tile.TileContext(nc) as tc:
        with (
            tc.tile_pool(name="const", bufs=1) as pconst,
            tc.tile_pool(name="stat", bufs=1) as pstat,
            tc.tile_pool(name="sраw", bufs=1) as praw,
            tc.tile_pool(name="st", bufs=2 * KC) as pst,
            tc.tile_pool(name="qt", bufs=1) as pqt,
            tc.tile_pool(name="qc", bufs=1) as pqc,
            tc.tile_pool(name="sq", bufs=3) as psq,
            tc.tile_pool(name="nrm", bufs=2) as pnrm,
            tc.tile_pool(name="best", bufs=1) as pbest,
            tc.tile_pool(name="res", bufs=2) as pres,
            tc.tile_pool(name="n2psum", bufs=2, space="PSUM") as pn2,
            tc.tile_pool(name="mmpsum", bufs=5, space="PSUM") as pmm,
        ):
            ones_sb = pconst.tile([128, 128], F32, tag="ones")
            nc.sync.dma_start(ones_sb[:], ones_cr[:])

            # ---- loads: queries first (small), then 4 support d-chunks
            qt_tiles = []
            with nc.named_scope("load_q"):
                for k in range(KC):
                    qt_ = pqt.tile([128, QSP], F32, name=f"qt{k}", tag=f"qt{k}")
                    nc.sync.dma_start(qt_[:], qry_t[k * 128:(k + 1) * 128, :])
                    qt_tiles.append(qt_)
            st_raw = []
            with nc.named_scope("load_s"):
                for k in range(KC):
                    st_ = praw.tile([128, CS], F32, name=f"sraw{k}", tag=f"sraw{k}")
                    nc.sync.dma_start(st_[:], sup_t[k * 128:(k + 1) * 128, :])
                    st_raw.append(st_)

            # ---- mean: free-dim reduces pipelined under the DMA
            with nc.named_scope("mean"):
                msub = pstat.tile([128, KC * MR], F32, tag="msub")
                for k in range(KC):
                    for m in range(MR):
                        sl = st_raw[k][:, m * MW:(m + 1) * MW]
                        col = msub[:, k * MR + m:k * MR + m + 1]
                        if m % 2 == 0:
                            nc.vector.tensor_reduce(
                                out=col, in_=sl, axis=mybir.AxisListType.X,
                                op=AluOpType.add)
                        else:
                            dump = psq.tile([128, MW], F32, tag="mdump")
                            nc.scalar.activation(dump[:], sl, AF.Copy,
                                                 accum_out=col)
                mu_t = pstat.tile([128, KC], F32, tag="mu_t")
                nmu_t = pstat.tile([128, KC], F32, tag="nmu_t")
                for k in range(KC):
                    acc = pstat.tile([128, 1], F32, tag=f"macc{k}")
                    nc.vector.tensor_reduce(
                        out=acc[:], in_=msub[:, k * MR:(k + 1) * MR],
                        axis=mybir.AxisListType.X, op=AluOpType.add)
                    nc.vector.tensor_scalar_mul(mu_t[:, k:k + 1], acc[:],
                                                1.0 / CS)
                    nc.vector.tensor_scalar_mul(nmu_t[:, k:k + 1], acc[:],
                                                -1.0 / CS)

            # ---- query side: center exactly, split into bf16 hi/lo
            qhi_tiles, qlo_tiles = [], []
            with nc.named_scope("qside"):
                for k in range(KC):
                    qc = pqc.tile([128, QSP], F32, name=f"qc{k}", tag=f"qc{k}")
                    nc.vector.tensor_scalar_sub(qc[:], qt_tiles[k][:],
                                                mu_t[:, k:k + 1])
                    qhi = pqc.tile([128, QSP], BF16, name=f"qhi{k}",
                                   tag=f"qhi{k}")
                    nc.scalar.copy(qhi[:], qc[:])
                    qlo = pqc.tile([128, QSP], BF16, name=f"qlo{k}",
                                   tag=f"qlo{k}")
                    nc.vector.tensor_sub(qlo[:], qc[:], qhi[:])
                    qhi_tiles.append(qhi)
                    qlo_tiles.append(qlo)

            # ---- pipelined per cs-chunk: norms -> normalize -> matmul
            best_tiles = [pbest.tile([PW, C], F32, name=f"best{i}", tag=f"best{i}")
                          for i in range(QT)]
            for j in range(NJ):
                cs0 = j * CSCH
                with nc.named_scope(f"prep{j}"):
                    # |s - mu|^2 via Square(x + (-mu)) then f32r colsum,
                    # replicated across all 128 partitions by the ones lhsT
                    n2_ps = pn2.tile([128, CSCH], F32, tag="n2")
                    for k in range(KC):
                        sqj = psq.tile([128, CSCH], F32, tag="sq")
                        nc.scalar.activation(
                            sqj[:], st_raw[k][:, cs0:cs0 + CSCH],
                            AF.Square, bias=nmu_t[:, k:k + 1])
                        nc.tensor.matmul(n2_ps[:], ones_sb[:], sqj[:],
                                         start=(k == 0), stop=(k == KC - 1))
                    nrm = pnrm.tile([128, CSCH], F32, tag="nrm")
                    nc.scalar.activation(nrm[:], n2_ps[:], AF.Sqrt)
                    inv = pnrm.tile([128, CSCH], F32, tag="inv")
                    rscr = pnrm.tile([128, CSCH], F32, tag="rscr")
                    nc.vector.reciprocal_approx_accurate(inv[:], nrm[:],
                                                         rscr[:])
                    # normalized support chunk: (x - mu) * inv, one fused
                    # pass, then split into bf16 hi/lo for exact 3-pass mains
                    shi, slo = [], []
                    for k in range(KC):
                        st_ = pst.tile([128, CSCH], F32, name=f"st{k}_{j}",
                                       tag="st")
                        nc.vector.scalar_tensor_tensor(
                            out=st_[:], in0=st_raw[k][:, cs0:cs0 + CSCH],
                            scalar=mu_t[:, k:k + 1], in1=inv[:],
                            op0=AluOpType.subtract, op1=AluOpType.mult)
                        hi = pst.tile([128, CSCH], BF16, name=f"shi{k}_{j}",
                                      tag="shi")
                        nc.scalar.copy(hi[:], st_[:])
                        lo = pst.tile([128, CSCH], BF16, name=f"slo{k}_{j}",
                                      tag="slo")
                        nc.vector.tensor_sub(lo[:], st_[:], hi[:])
                        shi.append(hi)
                        slo.append(lo)
                with nc.named_scope(f"mm{j}"):
                    ps_list = [pmm.tile([PW, CSCH], F32, tag="sims",
                                        name=f"ps{i}_{j}")
                               for i in range(QT)]
                    nmm = 3 * KC
                    pidx = 0
                    for k in range(KC):
                        for hsel, ssel in ((qhi_tiles, shi), (qhi_tiles, slo),
                                           (qlo_tiles, shi)):
                            for i in range(QT):
                                nc.tensor.matmul(
                                    ps_list[i][:],
                                    hsel[k][:, i * PW:(i + 1) * PW],
                                    ssel[k][:],
                                    start=(pidx == 0), stop=(pidx == nmm - 1),
                                )
                            pidx += 1
                    for i in range(QT):
                        nc.vector.tensor_reduce(
                            out=best_tiles[i][:, j * GPC:(j + 1) * GPC],
                            in_=ps_list[i][:].rearrange("p (c s) -> p c s", s=S),
                            axis=mybir.AxisListType.X, op=AluOpType.max,
                        )

            # ---- argmax over classes
            with nc.named_scope("argmax"):
                for i in range(QT):
                    valid = min(PW, QS - i * PW)
                    mx8 = pres.tile([PW, 8], F32, tag="mx8")
                    ix8 = pres.tile([PW, 8], U32, tag="ix8")
                    nc.vector.max_with_indices(mx8[:], ix8[:], best_tiles[i][:])
                    ii = pres.tile([PW, 1], I32, tag="ii")
                    nc.vector.tensor_copy(ii[:], ix8[:, 0:1])
                    nc.sync.dma_start(out[i * PW:i * PW + valid, :],
                                      ii[0:valid, :])

    nc.finalize()
    return nc


def _host_inputs(support_features, query_features):
    sup = np.asarray(support_features, dtype=np.float32).reshape(CS, D)
    sup_t = np.ascontiguousarray(sup.T)
    qf = np.asarray(query_features, dtype=np.float32)
    ones_cr = np.ones((128, 128), dtype=np.float32)
    in_maps = []
    for c in range(NCORES):
        qslab = np.zeros((QSP, D), dtype=np.float32)
        qslab[:QS] = qf[c * QS:(c + 1) * QS]
        in_maps.append({
            "support_t": sup_t,
            "queries_t": np.ascontiguousarray(qslab.T),
            "ones_cr": ones_cr,
        })
    return in_maps


def run(support_features, query_features, trace=False, **trace_kwargs):
    nc = build()
    in_maps = _host_inputs(support_features, query_features)
    res = run_bass_kernel_spmd(nc, in_maps, list(range(NCORES)),
                               trace=trace, **trace_kwargs)
    outs = [np.asarray(r["out"]).reshape(QS) for r in res.results]
    return np.concatenate(outs).astype(np.int32), res


def kernel(support_features, query_features, use_cosine=None, **_ignored):
    # use_cosine does not change the result: with L2-normalized vectors the
    # euclidean argmin equals the cosine argmax (monotone map), so one kernel
    # serves both branches.
    out, _ = run(support_features, query_features, trace=False)
    return out
